# revision 1
# baseline (speedup 1.0000x reference)
"""BaseGCN (4-layer GCN + mean-pool + MLP) on 8 Trainium2 NeuronCores.

Strategy: dst-sharded graph parallel, GPSIMD ap_gather message gathering,
PE one-hot scatter matmuls.

  - z tables live in SBUF feature-major: [128 partitions, NEL] where a
    partition-row group holds one src chunk's features.  Layers 1/2/4
    aggregate at width <= 8 and use EIGHT chunks (chunk = src core, 16
    partition rows each) so each gather column serves 8 edges; layer 3
    (width 32) uses FOUR chunks (32 rows each, 2 cores per chunk).
  - dst nodes are packed into "bins" (<= 64 nodes for the 8-chunk group,
    <= 32 for the 4-chunk group) such that every (bin, chunk) has <= 128
    edges; a bin owns one 128-column block of the gather stream, shared
    by all chunks.
  - GPSIMD ap_gather pulls per-edge columns feature-major; PE transposes
    128-col slabs to edge-major; K=128 matmuls against streamed one-hot
    tiles accumulate agg^T [w, 512] per bank in PSUM.
  - Weights/bias/PReLU applied in transposed space; z^T written per-bank
    to DRAM; AllGather exchanges z^T between layers.
  - Layer 4 aggregates z4 = h3 @ (W4 lw1 lw2) at width 4 reusing the
    8-chunk streams, then pools via per-bank transposes + matmuls against
    a static [slot, graph] 1/cnt matrix; AllReduce + constant finishes.

GCNConv(x) = A_hat (x W) + b with A_hat = D^-1/2 A D^-1/2 + D^-1 I;
aggregation commutes with the weight matmul so we aggregate at
width min(d_in, d_out): widths 8, 8, 32, 4.
"""

import os
import numpy as np

# ---------------- problem constants (hardcoded per the contract) ----------
N = 100000
E = 1600000
B = 64
NC = 8
NPC = N // NC          # 12500 dst nodes per core
CAP = 128              # edges per (bin, chunk) == columns per bin
BIN8 = 64              # node columns per bin, 8-chunk group
BIN4 = 32              # node columns per bin, 4-chunk group
NI8 = 1024             # gather columns per bank, 8-chunk group (8 bins)
NI4 = 2048             # gather columns per bank, 4-chunk group (16 bins)
D_IN = 8
DIMS = [8, 32, 64]             # h widths for layers 1..3
AGG_W = [8, 8, 32, 4]          # aggregation widths per layer
F32 = np.float32


def _bin_nodes(sizes, maxn, group):
    """Balanced multiway packing: fix a target bin count, then place each
    node (desc by load) into the feasible bin minimizing the resulting max
    chunk load.  sizes: [n, CH] int.  Returns (bin_of, col_of, n_bins)."""
    n, ch = sizes.shape
    tot = sizes.sum(1)
    target = max(int(np.ceil(n / maxn)),
                 int(np.ceil(sizes.sum(0).max() * 1.035 / CAP)),
                 int(np.ceil(sizes.sum() * 1.03 / (ch * CAP))))
    target = int(np.ceil(target / group) * group)
    order = np.argsort(-tot, kind="stable")
    cap_bins = target + 4 * group
    loads = np.zeros((cap_bins, ch), np.int64)
    cnts = np.zeros(cap_bins, np.int64)
    nbins = target
    bin_of = np.zeros(n, np.int32)
    col_of = np.zeros(n, np.int32)
    for v in order:
        s = sizes[v]
        cand = loads[:nbins] + s
        mx = cand.max(1)
        feas = (cand <= CAP).all(1) & (cnts[:nbins] < maxn)
        if feas.any():
            mx[~feas] = 1 << 30
            bi = int(np.argmin(mx))
        else:
            bi = nbins
            nbins += 1
        bin_of[v] = bi
        col_of[v] = cnts[bi]
        loads[bi] += s
        cnts[bi] += 1
    return bin_of, col_of, nbins


def _preprocess(x, edge_index, batch):
    src = edge_index[0].astype(np.int64)
    dst = edge_index[1].astype(np.int64)
    batch = batch.astype(np.int64)

    deg = np.bincount(dst, minlength=N).astype(F32) + 1.0
    dinv = (1.0 / np.sqrt(deg)).astype(F32)
    dinv2 = (dinv * dinv).astype(F32)

    # edge-only streams for the 8-chunk group (self loops handled via
    # algebraic diagonal terms); self loops stay in-stream for layer 3.
    allsrc = np.concatenate([src, np.arange(N, dtype=np.int64)])
    alldst = np.concatenate([dst, np.arange(N, dtype=np.int64)])
    allval = np.concatenate([dinv[src] * dinv[dst], dinv2]).astype(F32)
    is_edge = np.concatenate([np.ones(len(src), bool), np.zeros(N, bool)])

    cnt = np.maximum(np.bincount(batch, minlength=B).astype(F32), 1.0)

    sc8 = (allsrc // NPC).astype(np.int64)            # src core = 8-chunk id
    sc4 = sc8 // 2                                    # 4-chunk id
    e_core = (alldst // NPC).astype(np.int64)

    # ---- per-core binning (edges only; one bin structure for all layers) --
    bin8_of = np.zeros(N, np.int32); col8_of = np.zeros(N, np.int32)
    nb8 = []
    for c in range(NC):
        lo, hi = c * NPC, (c + 1) * NPC
        me = (alldst >= lo) & (alldst < hi) & is_edge
        d8 = np.zeros((NPC, 8), np.int64)
        np.add.at(d8, (alldst[me] - lo, sc8[me]), 1)
        b_o, c_o, nb = _bin_nodes(d8, BIN8, 8)
        bin8_of[lo:hi] = b_o; col8_of[lo:hi] = c_o; nb8.append(nb)
    NQB = int(np.ceil(max(nb8) / 8) * 8)         # bins count (x8 per bank)
    B8 = NQB // 8
    QS = NQB * BIN8                              # qs slots per core
    assert QS <= 16384, QS

    qs_slot = (bin8_of.astype(np.int64) * BIN8 + col8_of)   # core-local

    OHW8 = 8 * 8 * BIN8      # 4096
    idx8 = np.zeros((NC, B8, 128, NI8 // 16), np.int16)
    oh8 = np.zeros((NC, B8, 128, OHW8), np.float16)

    for c in range(NC):
        m = (e_core == c) & is_edge
        ev = allval[m]
        esrc = allsrc[m]
        edst = alldst[m]
        esc8 = sc8[m]
        key = bin8_of[edst].astype(np.int64) * 8 + esc8
        order = np.argsort(key, kind="stable")
        ks = key[order]
        starts = np.searchsorted(ks, np.arange(NQB * 8))
        pos = np.arange(len(ks)) - starts[ks]
        assert pos.max() < CAP
        bn, ch = ks // 8, ks % 8
        col = bn * CAP + pos
        bank, cb = col // NI8, col % NI8
        idx8[c, bank, 16 * ch + cb % 16, cb // 16] = qs_slot[esrc[order]]
        blk = (cb // 128)
        oh8[c, bank, cb % 128,
            (blk * 8 + ch) * BIN8 + col8_of[edst[order]]] = ev[order]

    # layer-4 pooling one-hots in QS space: [B8 banks, 128, 4, B] fp16
    poolw = np.zeros((NC, B8, 128, 4, B), np.float16)    # 1/cnt
    pool2w = np.zeros((NC, B8, 128, 4, B), np.float16)   # dinv2/cnt (selfs)
    # xself: x * dinv2 in QS columns (layer-1 diagonal term)
    xself = np.zeros((NC, 8, QS), F32)
    # dinv2 in QS columns (layer-2/3 diagonal terms); rows 0:16 and 32:48
    # carry dinv2 for the padded 48-row layer-3 layout, middle rows zero
    d2q = np.zeros((NC, 48, QS), F32)
    # x in 8-chunk table layout [128, QS]
    xtab = np.zeros((128, QS), F32)
    for c in range(NC):
        nodes = np.arange(c * NPC, (c + 1) * NPC)
        s = qs_slot[nodes]
        g = batch[nodes]
        poolw[c, s // 512, s % 128, (s % 512) // 128, g] = \
            (1.0 / cnt[g]).astype(np.float16)
        pool2w[c, s // 512, s % 128, (s % 512) // 128, g] = \
            (dinv2[nodes] / cnt[g]).astype(np.float16)
        xself[c, :, s] = x[nodes] * dinv2[nodes][:, None]
        d2q[c, :16][:, s] = dinv2[nodes][None, :].repeat(16, 0)
        d2q[c, 32:48][:, s] = dinv2[nodes][None, :].repeat(16, 0)
        xtab[16 * c:16 * c + D_IN, s] = x[nodes].T

    cfg = dict(B8=B8, QS=QS)
    return cfg, xtab, idx8, oh8, poolw, pool2w, xself, d2q


def _build_program(cfg):
    import concourse.bacc as bacc
    import concourse.tile as tile
    import concourse.bass as bass
    import concourse.mybir as mybir
    from concourse.masks import make_identity
    from contextlib import ExitStack

    dt = mybir.dt
    B8, QS = cfg["B8"], cfg["QS"]
    OHW8 = 8 * 8 * BIN8

    nc = bacc.Bacc("TRN2", target_bir_lowering=False, debug=False, num_devices=NC)

    xtab_d = nc.dram_tensor("xtab", [128, QS], dt.float32, kind="ExternalInput")
    idx8_d = nc.dram_tensor("idx8", [B8, 128, NI8 // 16], dt.int16, kind="ExternalInput")
    oh8_d = nc.dram_tensor("oh8", [B8, 128, OHW8], dt.float16, kind="ExternalInput")
    poolw_d = nc.dram_tensor("poolw", [B8, 128, 4, B], dt.float16,
                             kind="ExternalInput")
    pool2w_d = nc.dram_tensor("pool2w", [B8, 128, 4, B], dt.float16,
                              kind="ExternalInput")
    xself_d = nc.dram_tensor("xself", [8, QS], dt.float32, kind="ExternalInput")
    d2q_d = nc.dram_tensor("d2q", [48, QS], dt.float32, kind="ExternalInput")
    Wd = {}
    for i, (ki, ko) in enumerate([(8, 8), (8, 32), (48, 64), (64, 4)]):
        Wd[i] = nc.dram_tensor(f"W{i+1}", [ki, ko], dt.float32, kind="ExternalInput")
    bd, ad = {}, {}
    for i, d in enumerate(DIMS):
        bd[i] = nc.dram_tensor(f"b{i+1}", [d, 1], dt.float32, kind="ExternalInput")
        ad[i] = nc.dram_tensor(f"a{i+1}", [d, 1], dt.float32, kind="ExternalInput")
    cvec_d = nc.dram_tensor("cvec", [4, 1], dt.float32, kind="ExternalInput")
    out_d = nc.dram_tensor("out", [4, B], dt.float32, kind="ExternalOutput")

    AG = mybir.AluOpType

    with tile.TileContext(nc) as tc, ExitStack() as ctx:
        wpool = ctx.enter_context(tc.tile_pool(name="weights", bufs=1))
        dram = ctx.enter_context(tc.tile_pool(name="dram", bufs=1, space="DRAM"))
        sb = ctx.enter_context(tc.tile_pool(name="sb", bufs=3))
        sbB = ctx.enter_context(tc.tile_pool(name="sbB", bufs=2))
        psA = ctx.enter_context(tc.tile_pool(name="psA", bufs=2, space="PSUM"))
        psB = ctx.enter_context(tc.tile_pool(name="psB", bufs=1, space="PSUM"))
        psC = ctx.enter_context(tc.tile_pool(name="psC", bufs=1, space="PSUM"))
        psT = ctx.enter_context(tc.tile_pool(name="psT", bufs=2, space="PSUM"))
        psT2 = ctx.enter_context(tc.tile_pool(name="psT2", bufs=1, space="PSUM"))
        psP = ctx.enter_context(tc.tile_pool(name="psP", bufs=1, space="PSUM"))

        table = wpool.tile([128, 2 * QS], dt.float32, name="table")
        ident = wpool.tile([128, 128], dt.float32, name="ident")
        make_identity(nc, ident[:])

        Wt, bt, at = {}, {}, {}
        for i, (ki, ko) in enumerate([(8, 8), (8, 32), (48, 64), (64, 4)]):
            Wt[i] = wpool.tile([ki, ko], dt.float32, tag=f"w{i}", name=f"wt{i}")
            nc.sync.dma_start(Wt[i][:], Wd[i][:])
        for i, d in enumerate(DIMS):
            bt[i] = wpool.tile([d, 1], dt.float32, tag=f"b{i}", name=f"bt{i}")
            nc.sync.dma_start(bt[i][:], bd[i][:])
            at[i] = wpool.tile([d, 1], dt.float32, tag=f"a{i}", name=f"at{i}")
            nc.sync.dma_start(at[i][:], ad[i][:])
        cvt = wpool.tile([4, 1], dt.float32, name="cvt")
        nc.sync.dma_start(cvt[:], cvec_d[:])

        zownT = {1: dram.tile([8, QS], dt.float32, name="zo1"),
                 2: dram.tile([32, QS], dt.float32, name="zo2"),
                 3: dram.tile([4, QS], dt.float32, name="zo3")}
        zfullT = {1: dram.tile([NC, 8, QS], dt.float32, name="zf1"),
                  2: dram.tile([NC, 32, QS], dt.float32, name="zf2"),
                  3: dram.tile([NC, 4, QS], dt.float32, name="zf3")}
        pool_in = dram.tile([4, B], dt.float32, name="pin")
        pool_out = dram.tile([4, B], dt.float32, name="pout")

        def agg_phase(nel, tab_offs, w, nbanks, body):
            """Gather+transpose+scatter for nbanks banks over the 8-chunk
            streams.  One gather per table half in tab_offs; body(bank, h, t,
            c, lhsT_ap, oh_tile) emits one K=128 matmul per (block, chunk)."""
            for s in range(nbanks):
                idx_t = sb.tile([128, NI8 // 16], dt.int16, tag="idx", name="idx")
                nc.sync.dma_start(idx_t[:], idx8_d[s])
                oh_t = sb.tile([128, OHW8], dt.float16, tag="oh", name="oh")
                nc.sync.dma_start(oh_t[:], oh8_d[s])
                for h, off in enumerate(tab_offs):
                    msgT = sb.tile([128, NI8], dt.float32, tag="msg", name="msg")
                    nc.gpsimd.ap_gather(msgT[:], table[:, off:off + nel],
                                        idx_t[:], channels=128, num_elems=nel,
                                        d=1, num_idxs=NI8)
                    for sg in range(2):
                        trp = psT.tile([128, 512], dt.float32, tag="trp",
                                       name="trp")
                        for jp in range(4):
                            nc.tensor.transpose(
                                trp[:, jp * 128:jp * 128 + 128],
                                msgT[:, 128 * (sg * 4 + jp):128 * (sg * 4 + jp) + 128],
                                ident[:])
                        slabs = sbB.tile([128, 512], dt.float16, tag="slabs",
                                         name="slabs")
                        nc.vector.tensor_copy(slabs[:], trp[:])
                        for jp in range(4):
                            t = sg * 4 + jp
                            for c in range(8):
                                body(s, h, t, c,
                                     slabs[:, jp * 128 + 16 * c:jp * 128 + 16 * c + w],
                                     oh_t)

        def layer(l):  # l = 0, 1, 2
            w = AGG_W[l]
            d = DIMS[l]
            if l == 0:
                nc.scalar.dma_start(table[:, 0:QS], xtab_d[:])
            elif l == 1:
                for c in range(NC):
                    nc.scalar.dma_start(table[16 * c:16 * c + 8, 0:QS],
                                        zfullT[1][c])
            else:
                for c in range(NC):
                    nc.scalar.dma_start(table[16 * c:16 * c + 16, 0:QS],
                                        zfullT[2][c, 0:16])
                    nc.scalar.dma_start(table[16 * c:16 * c + 16, QS:2 * QS],
                                        zfullT[2][c, 16:32])

            state = {}

            wh = 16 if l == 2 else w

            def body(bank, h, t, c, lhsT, oh_t):
                if h == 0 and t == 0 and c == 0:
                    state["agg"] = psA.tile([64, 512], dt.float32, tag="agg",
                                            name="agg")
                nc.tensor.matmul(
                    state["agg"][32 * h:32 * h + wh,
                                 BIN8 * t:BIN8 * t + BIN8],
                    lhsT=lhsT,
                    rhs=oh_t[:, (t * 8 + c) * BIN8:(t * 8 + c + 1) * BIN8],
                    start=(c == 0), stop=(c == 7))
                if l == 2:
                    done = (h == 1 and t == 7 and c == 7)
                else:
                    done = (t == 7 and c == 7)
                if done:
                    bphase(bank, state["agg"])

            def bphase(bank, agg_ps):
                aggs = sbB.tile([48 if l == 2 else w, 512], dt.float32,
                                tag="aggs", name="aggs")
                if l == 0:
                    xs = sb.tile([8, 512], dt.float32, tag="xs", name="xs")
                    nc.sync.dma_start(xs[:], xself_d[:, 512 * bank:512 * bank + 512])
                    nc.vector.tensor_add(aggs[:], agg_ps[0:8, :], xs[:])
                elif l == 1:
                    zs = sb.tile([8, 512], dt.float32, tag="xs", name="zs")
                    nc.sync.dma_start(zs[:], zownT[1][:, 512 * bank:512 * bank + 512])
                    d2 = sb.tile([8, 512], dt.float32, tag="d2", name="d2")
                    nc.sync.dma_start(d2[:], d2q_d[0:8, 512 * bank:512 * bank + 512])
                    zsd = sbB.tile([8, 512], dt.float32, tag="zsd", name="zsd")
                    nc.vector.tensor_mul(zsd[:], zs[:], d2[:])
                    nc.vector.tensor_add(aggs[:], agg_ps[0:8, :], zsd[:])
                else:
                    # halves live at partitions 0:16 and 32:48; contract with
                    # a host-padded [48, 64] W3 (rows 16:32 zero)
                    zs = sb.tile([48, 512], dt.float32, tag="xs3", name="zs3")
                    nc.sync.dma_start(zs[0:16, :],
                                      zownT[2][0:16, 512 * bank:512 * bank + 512])
                    nc.sync.dma_start(zs[32:48, :],
                                      zownT[2][16:32, 512 * bank:512 * bank + 512])
                    d2 = sb.tile([48, 512], dt.float32, tag="d23", name="d23")
                    nc.sync.dma_start(d2[:], d2q_d[:, 512 * bank:512 * bank + 512])
                    zsd = sbB.tile([48, 512], dt.float32, tag="zsd3", name="zsd3")
                    nc.vector.tensor_mul(zsd[:], zs[:], d2[:])
                    nc.vector.memset(aggs[:], 0.0)
                    nc.vector.tensor_add(aggs[0:16, :], agg_ps[0:16, :],
                                         zsd[0:16, :])
                    nc.vector.tensor_add(aggs[32:48, :], agg_ps[32:48, :],
                                         zsd[32:48, :])
                h_ps = psB.tile([d, 512], dt.float32, tag="h", name="h")
                nc.tensor.matmul(h_ps[:], lhsT=Wt[l][:], rhs=aggs[:],
                                 start=True, stop=True)
                neg = sbB.tile([d, 512], dt.float32, tag="neg", name="neg")
                nc.vector.tensor_scalar(neg[:], h_ps[:], bt[l][:], 0.0, AG.add, AG.min)
                nega = sbB.tile([d, 512], dt.float32, tag="nega", name="nega")
                nc.vector.tensor_scalar(nega[:], neg[:], at[l][:], None, AG.mult)
                pos = sbB.tile([d, 512], dt.float32, tag="pos", name="pos")
                nc.vector.tensor_scalar(pos[:], h_ps[:], bt[l][:], 0.0, AG.add, AG.max)
                hT = sbB.tile([d, 512], dt.float32, tag="hT", name="hT")
                nc.vector.tensor_add(hT[:], pos[:], nega[:])
                if l == 2:
                    z4_ps = psC.tile([4, 512], dt.float32, tag="z4", name="z4")
                    nc.tensor.matmul(z4_ps[:], lhsT=Wt[3][:], rhs=hT[:],
                                     start=True, stop=True)
                    z4s = sbB.tile([4, 512], dt.float32, tag="z4s", name="z4s")
                    nc.vector.tensor_copy(z4s[:], z4_ps[:])
                    nc.scalar.dma_start(zownT[3][:, 512 * bank:512 * bank + 512],
                                        z4s[:])
                    # layer-4 self-loop term: pool dinv2*z4 directly
                    p2 = sb.tile([128, 4, B], dt.float16, tag="pw", name="p2w")
                    nc.sync.dma_start(p2[:], pool2w_d[bank])
                    for j in range(4):
                        trp2 = psT2.tile([128, 4], dt.float32, tag="trp2",
                                         name="trp2")
                        nc.tensor.transpose(trp2[:],
                                            z4s[:, 128 * j:128 * j + 128],
                                            ident[:4, :4])
                        trs2 = sbB.tile([128, 4], dt.float16, tag="trs",
                                        name="trs2")
                        nc.vector.tensor_copy(trs2[:], trp2[:])
                        nc.tensor.matmul(pool_ps[:],
                                         lhsT=trs2[:], rhs=p2[:, j, :],
                                         start=(bank == 0 and j == 0),
                                         stop=False)
                else:
                    nc.scalar.dma_start(
                        zownT[l + 1][:, 512 * bank:512 * bank + 512], hT[:])

            if l == 2:
                agg_phase(QS, [0, QS], 16, B8, body)
            else:
                agg_phase(QS, [0], w, B8, body)
            zkey = l + 1 if l < 2 else 3
            if os.environ.get("GCN_NO_CC"):
                nc.sync.dma_start(zfullT[zkey][0], zownT[zkey][:])
            else:
                nc.gpsimd.collective_compute(
                    "AllGather", AG.bypass, replica_groups=[list(range(NC))],
                    ins=[zownT[zkey][:].opt()], outs=[zfullT[zkey][:].opt()])

        pool_ps = psP.tile([4, B], dt.float32, name="pool_ps")

        for l in range(3):
            layer(l)

        # ---- layer 4: per-node agg of z4 (8-chunk streams), then pool ----
        for c in range(NC):
            nc.scalar.dma_start(table[16 * c:16 * c + 4, 0:QS], zfullT[3][c])
        state4 = {}

        def body4(bank, h, t, c, lhsT, oh_t):
            if t == 0 and c == 0:
                state4["agg"] = psA.tile([64, 512], dt.float32, tag="agg",
                                         name="agg4")
            nc.tensor.matmul(state4["agg"][0:4, BIN8 * t:BIN8 * t + BIN8],
                             lhsT=lhsT,
                             rhs=oh_t[:, (t * 8 + c) * BIN8:(t * 8 + c + 1) * BIN8],
                             start=(c == 0), stop=(c == 7))
            if t == NI8 // 128 - 1 and c == 7:
                pphase(bank, state4["agg"])

        def pphase(bank, agg_ps):
            aggs = sbB.tile([4, 512], dt.float32, tag="aggs", name="agg4s")
            nc.vector.tensor_copy(aggs[:], agg_ps[0:4, :])
            pw = sb.tile([128, 4, B], dt.float16, tag="pw", name="pw")
            nc.sync.dma_start(pw[:], poolw_d[bank])
            for j in range(4):
                trp = psT2.tile([128, 4], dt.float32, tag="trp2", name="trp4")
                nc.tensor.transpose(trp[:], aggs[:, 128 * j:128 * j + 128],
                                    ident[:4, :4])
                trs = sbB.tile([128, 4], dt.float16, tag="trs", name="trs")
                nc.vector.tensor_copy(trs[:], trp[:])
                nc.tensor.matmul(pool_ps[:],
                                 lhsT=trs[:], rhs=pw[:, j, :],
                                 start=False,
                                 stop=(bank == B8 - 1 and j == 3))

        agg_phase(QS, [0], 4, B8, body4)

        pooled = sbB.tile([4, B], dt.float32, name="pooled")
        nc.vector.tensor_copy(pooled[:], pool_ps[:])
        nc.sync.dma_start(pool_in[:], pooled[:])
        if os.environ.get("GCN_NO_CC"):
            nc.sync.dma_start(pool_out[:], pool_in[:])
        else:
            nc.gpsimd.collective_compute(
                "AllReduce", AG.add, replica_groups=[list(range(NC))],
                ins=[pool_in[:].opt()], outs=[pool_out[:].opt()])
        res = sbB.tile([4, B], dt.float32, name="res")
        nc.sync.dma_start(res[:], pool_out[:])
        res2 = sbB.tile([4, B], dt.float32, name="res2")
        nc.vector.tensor_scalar(res2[:], res[:], cvt[:], None, AG.add)
        nc.sync.dma_start(out_d[:], res2[:])

    nc.compile()
    return nc


def build(inputs):
    """Host preprocessing + program build. Returns (nc, in_maps)."""
    x = np.asarray(inputs["x"], F32)
    edge_index = np.asarray(inputs["edge_index"])
    batch = np.asarray(inputs["batch"])
    W = [np.asarray(inputs[f"W{i}"], F32) for i in range(1, 5)]
    b = [np.asarray(inputs[f"b{i}"], F32) for i in range(1, 5)]
    a = [np.asarray(inputs[f"a{i}"], F32) for i in range(1, 4)]
    lw1 = np.asarray(inputs["lw1"], F32)
    lb1 = np.asarray(inputs["lb1"], F32)
    lw2 = np.asarray(inputs["lw2"], F32)
    lb2 = np.asarray(inputs["lb2"], F32)

    (cfg, xtab, idx8, oh8, poolw, pool2w,
     xself, d2q) = _preprocess(x, edge_index, batch)

    W4p = (W[3] @ lw1 @ lw2).astype(F32)                     # [64, 4]
    W3p = np.zeros((48, 64), F32)                            # padded rows
    W3p[0:16] = W[2][0:16]
    W3p[32:48] = W[2][16:32]
    cv = (b[3] @ lw1 @ lw2 + lb1 @ lw2 + lb2).astype(F32)    # [4]

    nc = _build_program(cfg)

    in_maps = []
    for c in range(NC):
        m = dict(
            xtab=xtab, idx8=idx8[c], oh8=oh8[c],
            poolw=poolw[c], pool2w=pool2w[c], xself=xself[c], d2q=d2q[c],
            W1=W[0], W2=W[1], W3=W3p, W4=W4p,
            b1=b[0].reshape(-1, 1), b2=b[1].reshape(-1, 1), b3=b[2].reshape(-1, 1),
            a1=np.full((8, 1), a[0][0], F32),
            a2=np.full((32, 1), a[1][0], F32),
            a3=np.full((64, 1), a[2][0], F32),
            cvec=cv.reshape(4, 1),
        )
        in_maps.append(m)
    return nc, in_maps


def kernel(**inputs):
    nc, in_maps = build(inputs)
    from concourse.bass_utils import run_bass_kernel_spmd
    res = run_bass_kernel_spmd(nc, in_maps, list(range(NC)))
    outT = res.results[0]["out"]      # [4, B]
    return np.ascontiguousarray(outT.T.astype(F32))          # [B, 4]

